# revision 51
# baseline (speedup 1.0000x reference)
"""Exponential Hawkes process negative log-likelihood on 8 Trainium2 cores.

Math (reference):
    R_0 = 0;  R_i = exp(-beta*(t_i - t_{i-1})) * (1 + R_{i-1})
    lam_i = mu + alpha * R_i
    nll = -[ sum_i log(lam_i) - mu*T - (alpha/beta) * sum_i (1 - exp(-beta*(T - t_i)))
             - 1000 * relu(alpha/beta - 0.999)^2 ]

Strategy (blocked scan, per the sharding hint):
  - The serial bottleneck is the per-event affine recurrence.  The stock
    DVE ``tensor_tensor_scan`` runs ~2 cycles/element (state routed
    backward one stage through the ``out_a`` flop with a one-cycle
    bubble).  This kernel registers a hand-written custom DVE micro-op
    program with the bubble removed: elements issue every cycle and the
    backward route delivers the state from TWO elements back.
  - The recurrence is pair-compacted on the host: pair p has the affine
    map R -> A_p R + B_p over its two events.  On the pair sequence the
    custom op is fed an interleaved stream of quad maps (aligned pair
    (2r,2r+1) at even columns, straddled (2r-1,2r) at odd columns), so
    the 2-back recurrence yields the pair-chain state (= R at every odd
    event) at 1 element/cycle -- i.e. 0.5 DVE cycles per event.
  - Even events are recovered on-device with a second (elementwise)
    custom op, lam-arg xe_m = ae_m*(1 + s_{m-1}) fused as
    Src0*Src1 + Src0, reading the shifted state stream via strided APs.
  - The recovery ops write into the tail of the scan's own tile, so one
    contiguous ACT Ln(alpha*x + mu) with accum_out covers both streams
    per column tile (halves ACT instruction + accumulator-read count).
  - Events are sharded across 8 cores with a 1024-event halo (the carry
    decays to exactly 0 in f32 across it; verified against the data).
    Tiles > 0 re-scan a warmup window so chunk/tile truncation has
    decayed to 0 by the tile body; the first events of each chunk are
    recomputed with the exact carry on the host in f64.
  - The three bf16 input streams (3.15 MB/core -- vs 4.2 MB for raw f32
    event times) are prefetched into persistent SBUF via a few chunked
    DMAs issued up front, split across both DMA rings (Sync + Scalar) so
    input loads are never queued behind ACT work and outputs never
    behind input streams (the rings are in-order).
  - The integral tail sum and the final reduction are host-side f64.
"""

import numpy as np
import ml_dtypes

# Problem constants (hardcoded per task instructions).
N = 8_388_608          # total events
M = 8                  # cores
S = N // M             # events per shard (1,048,576)
H = 1024               # halo events prepended to each shard (tile-aligned)
L = S + H              # per-core sequence length
P = 128                # SBUF partitions
CE = L // P            # event columns per partition (8200)
C2 = CE // 2           # pair columns per partition (4100)
H2 = H // 2            # halo boundary in pair columns (512)
EPS = 1e-8
PENALTY = 1000.0
PAD_GAP = 1.0e6        # core-0 pad dt; exp(-beta*PAD_GAP) == 0 in f32

# Column tiles (start, width) on the pair-column grid; halo boundary (512)
# falls on a tile edge.  Tiles j>0 are scanned with a SKIP-column warmup
# prefix, so SKIP <= tiles[1][0] and SKIP <= tiles[0][1].
_TILES_A = [(0, 256), (256, 256), (512, 1664), (2176, 1280), (3456, 644)]
_TILES_B = [(0, 512), (512, 1536), (2048, 1024), (3072, 1028)]
assert sum(w for _, w in _TILES_A) == C2 and sum(w for _, w in _TILES_B) == C2
# input prefetch chunk boundaries (tile-start aligned)
_CHUNKS_A = [(0, 512), (512, 1664), (2176, 1280), (3456, 644)]
_CHUNKS_B = [(0, 512), (512, 1536), (2048, 2052)]

_PROGRAM_CACHE: dict = {}
_OP_CACHE: dict = {}


def _softplus64(x: float) -> float:
    return float(np.logaddexp(0.0, np.float64(x)))


def _get_custom_ops():
    """Register (once) the two custom DVE ops:

    AFFINE_SCAN_2BACK_ANT -- hand-written micro-op program:
        out[i] = in0[i] * state + in1[i],  state = out[i-2]  (seeded 0).
    Mirrors the stock tensor_tensor_scan datapath (stage 1 MULT via the
    backward NEXT_ALU_OUT_A route, stage 2 ADD writing the out_a state
    flop) minus the stock one-element bubble, so elements issue every
    cycle and the backward route delivers the state from 2 back.

    MULT_PLUS_SRC0_ANT -- DSL-lowered elementwise fused op:
        out[i] = in0[i]*in1[i] + in0[i]
    """
    if "ops" in _OP_CACHE:
        return _OP_CACHE["ops"]

    import concourse.dve_ops as dve_ops
    from concourse.dve_spec import Spec, Src0, Src1, C0, lower
    from concourse.dve_uop import (
        UopConfig, UopDpConfig, DveOpSpec, AluOp, AluInp, InpSel, OutSel,
        OutPath, Trigger, DelayInp,
    )

    ENABLE, DISABLE = 1, 0

    def _dp(stage: int, seed: bool) -> UopDpConfig:
        dp = UopDpConfig()
        dp.delay = [DelayInp.PREV_DELAY] * 4 + [DelayInp.PREV_ALU_OUT] * 3
        dp.delay_enable = [ENABLE] * 4 + [DISABLE] * 3
        dp.alu_out_enable = ENABLE
        if stage == 1 and not seed:
            dp.op = AluOp.MULTIPLY
            dp.alu_src0 = AluInp.NEXT_ALU_OUT_A   # state: stage 2's out_a flop
            dp.alu_src1 = AluInp.PREV_DELAY_0     # A_i
        elif stage == 2:
            if seed:
                dp.op = AluOp.BYPASS              # out/out_a <- 0 (lane 3)
                dp.alu_src0 = AluInp.PREV_DELAY_3
                dp.alu_src1 = AluInp.PREV_DELAY_3
            else:
                dp.op = AluOp.ADD                 # state*A + B
                dp.alu_src0 = AluInp.PREV_ALU_OUT
                dp.alu_src1 = AluInp.PREV_DELAY_2
            dp.alu_out_a_enable = ENABLE          # state write-back
        else:
            dp.op = AluOp.BYPASS
            dp.alu_src0 = AluInp.PREV_ALU_OUT
            dp.alu_src1 = AluInp.PREV_ALU_OUT
        return dp

    def _uop(seed: bool) -> UopConfig:
        u = UopConfig()
        u.inp = [InpSel.ZERO] * len(u.inp)
        u.inp_enable = [DISABLE] * len(u.inp_enable)
        u.inp[1], u.inp_enable[1] = InpSel.SRC_0, ENABLE    # lane 0 = A
        u.inp[3], u.inp_enable[3] = InpSel.SRC_1, ENABLE    # lane 2 = B
        u.inp[4], u.inp_enable[4] = InpSel.ZERO, ENABLE     # lane 3 = 0 seed
        u.out = {o: OutSel.ALU_OUT for o in OutPath}
        u.out_enable = {o: DISABLE for o in OutPath}
        if not seed:
            u.out_enable[OutPath.WR0_LO] = ENABLE
        u.datapath_config = [_dp(st, seed) for st in range(8)]
        if seed:
            # two non-consuming priming cycles write 0 into the state flop
            # (one per parity of the 2-back recurrence)
            u.repeat_count = 2
            u.trigger = (Trigger.COUNT, Trigger.NONE, Trigger.NONE)
            u.next_uop = (1, 0, 0)
            u.require_inp0 = DISABLE
            u.require_inp1 = DISABLE
        else:
            u.repeat_count = 0
            u.trigger = (Trigger.SRC_TENSOR_DONE, Trigger.NONE, Trigger.NONE)
            u.next_uop = (0, 0, 0)
            u.require_inp0 = ENABLE
            u.require_inp1 = ENABLE
        return u

    scan_uops = [_uop(seed=True), _uop(seed=False)]
    for u in scan_uops:
        u.validate("v3")

    class _Op:
        subdim = False
        perf_en: dict = {}

        def __init__(self, name, spec, uops):
            self.name = name
            self.spec = spec
            self._uops = uops

        def compile(self, ver):
            assert ver == "v3", f"custom ops only built for v3, got {ver}"
            import concourse.dve_ops as dve_ops
            key = ("compiled", self.name, ver)
            if key not in _OP_CACHE:
                uops = self._uops if self._uops is not None \
                    else lower(self.spec, ver=ver)
                _OP_CACHE[key] = DveOpSpec(
                    name=self.name,
                    opcode=dve_ops.get_dve_sub_opcode(self.name),
                    uops=uops,
                    rd1_en=True,
                )
            return _OP_CACHE[key]

    scan_op = _Op(
        "AFFINE_SCAN_2BACK_ANT",
        Spec(body=Src0 * C0 + Src1,
             reference=lambda in0, in1, s0, s1, imm2: in0 * s0 + in1),
        scan_uops,
    )
    fma_op = _Op(
        "MULT_PLUS_SRC0_ANT",
        Spec(body=Src0 * Src1 + Src0,
             reference=lambda in0, in1, s0, s1, imm2: in0 * in1 + in0),
        None,
    )
    for op in (scan_op, fma_op):
        if op.name not in dve_ops._SUB_OPCODE_FOR_NAME:
            dve_ops.OPS.append(op)
            dve_ops._SUB_OPCODE_FOR_NAME[op.name] = (
                dve_ops._CUSTOM_DVE_ROW_BASE + len(dve_ops.OPS) - 1)
            assert dve_ops._SUB_OPCODE_FOR_NAME[op.name] < 0x20
            dve_ops.CUSTOM_DVE_SPECS[op.name] = op.spec
    _OP_CACHE["ops"] = (scan_op, fma_op)
    return _OP_CACHE["ops"]


def _build_program(mu: float, alpha: float, tiles: tuple, chunks: tuple,
                   skip: int):
    import concourse.bacc as bacc
    import concourse.mybir as mybir
    from concourse.tile import TileContext

    f32 = mybir.dt.float32
    bf16 = mybir.dt.bfloat16
    AF = mybir.ActivationFunctionType
    NT = len(tiles)
    SK = skip
    assert 0 < SK <= tiles[0][1] and SK <= tiles[1][0] and SK % 2 == 0
    FMAX = 2 * max(w for _, w in tiles) + SK

    scan_op, fma_op = _get_custom_ops()

    nc = bacc.Bacc()
    in0 = nc.dram_tensor("in0", [P, C2], mybir.dt.float8e4, kind="ExternalInput")
    in1 = nc.dram_tensor("in1", [P, C2], mybir.dt.float8e4, kind="ExternalInput")
    fp8 = mybir.dt.float8e4
    aein = nc.dram_tensor("ae", [P, C2], fp8, kind="ExternalInput")
    # stats: per-tile states/merged Ln sums [0,NT), even-stream Ln sums
    # [NT,2NT) (only for split tiles), 2 end states
    out_stats = nc.dram_tensor("out_stats", [P, 2 * NT + 2], f32,
                               kind="ExternalOutput")
    out_bhead = nc.dram_tensor("out_bhead", [P, SK], f32,
                               kind="ExternalOutput")

    with TileContext(nc) as tc:
        with tc.tile_pool(name="pers", bufs=1) as pers, \
             tc.tile_pool(name="work", bufs=3) as work:
            stats = pers.tile([P, 2 * NT + 2], f32)
            musb = pers.tile([P, 1], f32)
            i0f = pers.tile([P, C2], mybir.dt.float8e4)
            i1f = pers.tile([P, C2], mybir.dt.float8e4)
            aef = pers.tile([P, C2], fp8)
            nc.gpsimd.memset(stats[:], 0.0)
            nc.gpsimd.memset(musb[:], float(mu))
            # prefetch the full input streams up front (chunked so early
            # tiles start while later chunks stream); in0/ae ride the Sync
            # ring, in1 the Scalar ring -- issued before any ACT work so
            # they are never queued behind it
            for c0, w in chunks:
                nc.sync.dma_start(i0f[:, c0:c0 + w], in0[:, c0:c0 + w])
                nc.scalar.dma_start(i1f[:, c0:c0 + w], in1[:, c0:c0 + w])
                nc.sync.dma_start(aef[:, c0:c0 + w], aein[:, c0:c0 + w])

            # dummy 1-element Ln: pulls the ~1.3us Ln ACT_TABLE_LOAD under
            # the input-DMA shadow instead of ahead of the first real Ln
            warm = pers.tile([P, 1], f32)
            nc.scalar.activation(warm[:], musb[:], AF.Ln,
                                 scale=float(alpha), bias=musb[:])

            for j, (c0, w) in enumerate(tiles):
                wu = 0 if j == 0 else SK         # warmup prefix columns
                a0 = c0 - wu
                wt = w + wu
                lo = SK if j == 0 else 0         # head skipped on device

                # bt: [0, wt) = scan states, [wt, wt+w) = even-event args
                bt = work.tile([P, FMAX], f32, tag="b")
                nc.vector._custom_dve(scan_op, out=bt[:, :wt],
                                      in0=i0f[:, a0:a0 + wt],
                                      in1=i1f[:, a0:a0 + wt])

                if j == 0:
                    # ship the carry-head block; its Ln is host-side
                    nc.scalar.dma_start(out_bhead[:], bt[:, :SK])

                # even-event lam args: xe_m = ae_m*(1 + s_{m-1}); the state
                # s_{m-1} sits at stream col m (m odd) / m-2 (m even)
                nc.vector._custom_dve(
                    fma_op, out=bt[:, wt + lo:wt + w:2],
                    in0=aef[:, c0 + lo:c0 + w:2],
                    in1=bt[:, wu + lo - 2:wu + w - 2:2])
                nc.vector._custom_dve(
                    fma_op, out=bt[:, wt + lo + 1:wt + w:2],
                    in0=aef[:, c0 + lo + 1:c0 + w:2],
                    in1=bt[:, wu + lo + 1:wu + w:2])

                lnl = work.tile([P, FMAX], bf16, tag="lnl")
                if j == 0 or w >= 1024:
                    # split: the states-half Ln depends only on the scan and
                    # overlaps the recovery on ACT; evens follow
                    nc.scalar.activation(lnl[:, :w - lo], bt[:, wu + lo:wt],
                                         AF.Ln, scale=float(alpha),
                                         bias=musb[:],
                                         accum_out=stats[:, j:j + 1])
                    nc.scalar.activation(lnl[:, w - lo:2 * (w - lo)],
                                         bt[:, wt + lo:wt + w],
                                         AF.Ln, scale=float(alpha),
                                         bias=musb[:],
                                         accum_out=stats[:, NT + j:NT + j + 1])
                else:
                    # states and even args are contiguous in bt: one Ln
                    nc.scalar.activation(lnl[:, :2 * w], bt[:, wu:wt + w],
                                         AF.Ln, scale=float(alpha),
                                         bias=musb[:],
                                         accum_out=stats[:, j:j + 1])

                if j == NT - 1:
                    # last two stream cols = states of pairs C2-1, C2-2
                    nc.vector.tensor_copy(stats[:, 2 * NT:2 * NT + 2],
                                          bt[:, wt - 2:wt])

            nc.scalar.dma_start(out_stats[:], stats[:], single_packet=True)

    nc.finalize()
    return nc


def _get_program(mu, alpha, tiles, chunks, skip):
    key = (repr(mu), repr(alpha), tuple(tiles), tuple(chunks), skip)
    prog = _PROGRAM_CACHE.get(key)
    if prog is None:
        prog = _build_program(mu, alpha, tiles, chunks, skip)
        _PROGRAM_CACHE[key] = prog
    return prog


def kernel(event_times, raw_mu, raw_alpha, raw_beta, _want_trace=False):
    from concourse.bass_utils import run_bass_kernel_spmd

    ev_full = np.ascontiguousarray(np.asarray(event_times, dtype=np.float32))
    assert ev_full.shape == (N,), ev_full.shape
    mu = _softplus64(float(np.asarray(raw_mu))) + EPS
    alpha = _softplus64(float(np.asarray(raw_alpha))) + EPS
    beta = _softplus64(float(np.asarray(raw_beta))) + EPS
    T = float(ev_full[-1])

    # a_i = exp(-beta*dt_i) over the halo-extended event array (f32 dt, f64
    # exp); index e in a_ext = global event e-H, the first H are core-0 pad.
    dt_full = np.empty(N, np.float64)
    dt_full[0] = PAD_GAP
    dt_full[1:] = np.subtract(ev_full[1:], ev_full[:-1], dtype=np.float32)
    a_ext = np.zeros(N + H, np.float64)
    np.exp(-beta * dt_full, out=a_ext[H:])

    # halo sufficiency: the carry truncated at each shard/halo start must
    # have decayed to 0 (in f32) before the first real event.
    halo_span = ev_full[np.arange(1, M) * S] - ev_full[np.arange(1, M) * S - H]
    if not np.all(beta * halo_span.astype(np.float64) > 120.0):
        raise RuntimeError(f"halo H={H} insufficient for beta={beta}")

    # Pair maps: pair q = events (2q, 2q+1):  R -> A_q R + B_q
    aep = a_ext[0::2]
    aop = a_ext[1::2]
    A = aep * aop
    B = A + aop
    # Quad-interleaved streams on the pair grid: even stream col 2r holds
    # the aligned quad (pairs 2r, 2r+1), odd col 2r+1 the straddled quad
    # (pairs 2r-1, 2r); the 2-back scan then emits the pair-chain state
    # (R at the pair's odd event) at every column, pairwise swapped.
    A_e, A_o = A[0::2], A[1::2]
    B_e, B_o = B[0::2], B[1::2]
    A_em1 = np.empty_like(A_e)                 # A[2r-1]
    A_em1[0], A_em1[1:] = 0.0, A_o[:-1]
    B_em1 = np.empty_like(B_e)                 # B[2r-1]
    B_em1[0], B_em1[1:] = 0.0, B_o[:-1]
    IN0 = np.empty(N + H >> 1, np.float32)
    IN1 = np.empty(N + H >> 1, np.float32)
    IN0[0::2] = A_e * A_o
    IN1[0::2] = A_o * B_e + B_o
    IN0[1::2] = A_e * A_em1
    IN1[1::2] = A_e * B_em1 + B_e
    IN0 = IN0.astype(ml_dtypes.float8_e4m3fn)
    IN1 = IN1.astype(ml_dtypes.float8_e4m3fn)
    AE16 = aep.astype(np.float32).astype(ml_dtypes.float8_e4m3fn)

    # Carry/warmup window: max number of events within 110/beta time units
    # ahead of any event (margin over the f32 exp underflow at ~104).
    cnt = (np.searchsorted(ev_full, ev_full + np.float32(110.0 / beta))
           - np.arange(N))
    wc_req = int(cnt.max())
    tiles, chunks = _TILES_A, _CHUNKS_A
    skip = min(-(-(wc_req + 96) // 64) * 32, tiles[0][1])
    if wc_req + 32 > 2 * skip or skip > tiles[1][0]:
        tiles, chunks = _TILES_B, _CHUNKS_B
        skip = min(-(-(wc_req + 96) // 64) * 32, tiles[0][1])
        if wc_req + 32 > 2 * skip or skip > tiles[1][0]:
            raise RuntimeError(
                f"carry window {wc_req} exceeds head tile; beta={beta} too "
                f"small for this build")

    # Per-core inputs and host-side fixup metadata
    S2, L2 = S // 2, L // 2
    in_maps = []
    t2ds = []      # per-core [P, CE] event-time windows (f64)
    for k in range(M):
        sl = slice(k * S2, k * S2 + L2)
        in_maps.append({
            "in0": np.ascontiguousarray(IN0[sl].reshape(P, C2)),
            "in1": np.ascontiguousarray(IN1[sl].reshape(P, C2)),
            "ae": np.ascontiguousarray(AE16[sl].reshape(P, C2)),
        })
        if k == 0:
            win_t = np.empty(L, np.float64)
            win_t[:H] = ev_full[0] - PAD_GAP
            win_t[H:] = ev_full[:S]
        else:
            win_t = ev_full[k * S - H:(k + 1) * S].astype(np.float64)
        t2ds.append(win_t.reshape(P, CE))

    prog = _get_program(mu, alpha, tuple(tiles), tuple(chunks), skip)
    res = run_bass_kernel_spmd(prog, in_maps, list(range(M)),
                               trace=_want_trace)

    NT = len(tiles)
    SK = skip
    q = np.arange(SK)
    qcol = q + np.where(q % 2 == 1, -1, 1)       # stream col of pair q
    log_term = np.float64(0.0)
    for k in range(M):
        r = res.results[k]
        st = r["out_stats"].astype(np.float64)   # [P, 2NT+2]
        lg = st[:, 0:2 * NT]
        for j, (c0, w) in enumerate(tiles):
            if c0 + w <= H2:     # partition-0 columns of this tile = halo
                lg[0, j] = 0.0
                lg[0, NT + j] = 0.0
        log_term += lg.sum()

        # Host-side carry (f64).  The device scanned each chunk's two
        # quad-parity chains with state 0 and a zeroed straddle stub, so
        # both chains' truncation is exp(-beta*(t_{2p+1} - t_{-1}))*K1
        # with K1 = R at the chunk's predecessor event; it has decayed to
        # exactly 0 (f32) for pairs >= SK into the chunk.
        t2d = t2ds[k]
        tp1 = np.empty(P, np.float64)   # t at row event -1
        tp1[0] = t2d[0, 0] - 1.0
        flat = t2d.reshape(-1)
        tp1[1:] = flat[CE - 1:L - 1:CE]
        bend1 = st[:, 2 * NT]        # state of pair C2-1 (odd pair chain)
        K1 = np.zeros(P, np.float64)
        r1 = 0.0
        for p in range(P):
            K1[p] = r1
            r1 = bend1[p] + np.exp(-beta * (t2d[p, CE - 1] - tp1[p])) * r1
        bhead = r["out_bhead"].astype(np.float64)    # [P, SK] stream cols
        # true pair states s(q), q in [0, SK)
        todd = t2d[:, 2 * q + 1]                     # [P, SK]
        s_true = (bhead[:, qcol]
                  + np.exp(-beta * (todd - tp1[:, None])) * K1[:, None])
        # odd events 2q+1
        ln_o = np.log(mu + alpha * s_true)
        # even events 2q: R = a*(1 + s(q-1))
        s_prev = np.empty_like(s_true)
        s_prev[:, 0] = K1
        s_prev[:, 1:] = s_true[:, :-1]
        teven = t2d[:, 2 * q]
        tprev = np.empty_like(teven)
        tprev[:, 0] = tp1
        tprev[:, 1:] = t2d[:, 1:2 * SK - 1:2]
        a_ev = np.exp(-beta * (teven - tprev))
        ln_e = np.log(mu + alpha * (a_ev * (1.0 + s_prev)))
        log_term += ln_o[1:, :].sum() + ln_e[1:, :].sum()   # row 0 = halo

    # Integral term fully on host (f64)
    lo_i = int(np.searchsorted(ev_full, np.float32(T - 700.0 / beta)))
    int_exp = float(np.exp(-beta * (np.float64(T) -
                                    ev_full[lo_i:].astype(np.float64))).sum())
    integral_term = mu * T + (alpha / beta) * (N - int_exp)

    branching = alpha / beta
    penalty = PENALTY * max(branching - 0.999, 0.0) ** 2
    loglik = log_term - integral_term - penalty
    out = np.float32(-loglik)
    if _want_trace:
        return out, res
    return out


# revision 52
# speedup vs baseline: 1.0577x; 1.0577x over previous
"""Exponential Hawkes process negative log-likelihood on 8 Trainium2 cores.

Math (reference):
    R_0 = 0;  R_i = exp(-beta*(t_i - t_{i-1})) * (1 + R_{i-1})
    lam_i = mu + alpha * R_i
    nll = -[ sum_i log(lam_i) - mu*T - (alpha/beta) * sum_i (1 - exp(-beta*(T - t_i)))
             - 1000 * relu(alpha/beta - 0.999)^2 ]

Strategy (blocked scan, per the sharding hint):
  - The serial bottleneck is the per-event affine recurrence.  The stock
    DVE ``tensor_tensor_scan`` runs ~2 cycles/element (state routed
    backward one stage through the ``out_a`` flop with a one-cycle
    bubble).  This kernel registers a hand-written custom DVE micro-op
    program with the bubble removed: elements issue every cycle and the
    backward route delivers the state from TWO elements back.
  - The recurrence is pair-compacted on the host: pair p has the affine
    map R -> A_p R + B_p over its two events.  On the pair sequence the
    custom op is fed an interleaved stream of quad maps (aligned pair
    (2r,2r+1) at even columns, straddled (2r-1,2r) at odd columns), so
    the 2-back recurrence yields the pair-chain state (= R at every odd
    event) at 1 element/cycle -- i.e. 0.5 DVE cycles per event.
  - Even events are recovered on-device with a second (elementwise)
    custom op, lam-arg xe_m = ae_m*(1 + s_{m-1}) fused as
    Src0*Src1 + Src0, reading the shifted state stream via strided APs.
  - The recovery ops write into the tail of the scan's own tile, so one
    contiguous ACT Ln(alpha*x + mu) with accum_out covers both streams
    per column tile (halves ACT instruction + accumulator-read count).
  - Events are sharded across 8 cores with a 1024-event halo (the carry
    decays to exactly 0 in f32 across it; verified against the data).
    Tiles > 0 re-scan a warmup window so chunk/tile truncation has
    decayed to 0 by the tile body; the first events of each chunk are
    recomputed with the exact carry on the host in f64.
  - The three bf16 input streams (3.15 MB/core -- vs 4.2 MB for raw f32
    event times) are prefetched into persistent SBUF via a few chunked
    DMAs issued up front, split across both DMA rings (Sync + Scalar) so
    input loads are never queued behind ACT work and outputs never
    behind input streams (the rings are in-order).
  - The integral tail sum and the final reduction are host-side f64.
"""

import numpy as np
import ml_dtypes

# Problem constants (hardcoded per task instructions).
N = 8_388_608          # total events
M = 8                  # cores
S = N // M             # events per shard (1,048,576)
H = 0                  # no halo: the host threads the carry chain across
                       # core boundaries exactly as across partition rows
L = S + H              # per-core sequence length
P = 128                # SBUF partitions
CE = L // P            # event columns per partition (8200)
C2 = CE // 2           # pair columns per partition (4100)
H2 = H // 2            # halo boundary in pair columns (512)
EPS = 1e-8
PENALTY = 1000.0
PAD_GAP = 1.0e6        # core-0 pad dt; exp(-beta*PAD_GAP) == 0 in f32

# Column tiles (start, width) on the pair-column grid; halo boundary (512)
# falls on a tile edge.  Tiles j>0 are scanned with a SKIP-column warmup
# prefix, so SKIP <= tiles[1][0] and SKIP <= tiles[0][1].
_TILES_A = [(0, 256), (256, 256), (512, 1664), (2176, 1280), (3456, 640)]
_TILES_B = [(0, 512), (512, 1536), (2048, 1024), (3072, 1024)]
assert sum(w for _, w in _TILES_A) == C2 and sum(w for _, w in _TILES_B) == C2
# input prefetch chunk boundaries (tile-start aligned)
_CHUNKS_A = [(0, 512), (512, 1664), (2176, 1280), (3456, 640)]
_CHUNKS_B = [(0, 512), (512, 1536), (2048, 2048)]

_PROGRAM_CACHE: dict = {}
_OP_CACHE: dict = {}


def _softplus64(x: float) -> float:
    return float(np.logaddexp(0.0, np.float64(x)))


def _get_custom_ops():
    """Register (once) the two custom DVE ops:

    AFFINE_SCAN_2BACK_ANT -- hand-written micro-op program:
        out[i] = in0[i] * state + in1[i],  state = out[i-2]  (seeded 0).
    Mirrors the stock tensor_tensor_scan datapath (stage 1 MULT via the
    backward NEXT_ALU_OUT_A route, stage 2 ADD writing the out_a state
    flop) minus the stock one-element bubble, so elements issue every
    cycle and the backward route delivers the state from 2 back.

    MULT_PLUS_SRC0_ANT -- DSL-lowered elementwise fused op:
        out[i] = in0[i]*in1[i] + in0[i]
    """
    if "ops" in _OP_CACHE:
        return _OP_CACHE["ops"]

    import concourse.dve_ops as dve_ops
    from concourse.dve_spec import Spec, Src0, Src1, C0, lower
    from concourse.dve_uop import (
        UopConfig, UopDpConfig, DveOpSpec, AluOp, AluInp, InpSel, OutSel,
        OutPath, Trigger, DelayInp,
    )

    ENABLE, DISABLE = 1, 0

    def _dp(stage: int, seed: bool) -> UopDpConfig:
        dp = UopDpConfig()
        dp.delay = [DelayInp.PREV_DELAY] * 4 + [DelayInp.PREV_ALU_OUT] * 3
        dp.delay_enable = [ENABLE] * 4 + [DISABLE] * 3
        dp.alu_out_enable = ENABLE
        if stage == 1 and not seed:
            dp.op = AluOp.MULTIPLY
            dp.alu_src0 = AluInp.NEXT_ALU_OUT_A   # state: stage 2's out_a flop
            dp.alu_src1 = AluInp.PREV_DELAY_0     # A_i
        elif stage == 2:
            if seed:
                dp.op = AluOp.BYPASS              # out/out_a <- 0 (lane 3)
                dp.alu_src0 = AluInp.PREV_DELAY_3
                dp.alu_src1 = AluInp.PREV_DELAY_3
            else:
                dp.op = AluOp.ADD                 # state*A + B
                dp.alu_src0 = AluInp.PREV_ALU_OUT
                dp.alu_src1 = AluInp.PREV_DELAY_2
            dp.alu_out_a_enable = ENABLE          # state write-back
        else:
            dp.op = AluOp.BYPASS
            dp.alu_src0 = AluInp.PREV_ALU_OUT
            dp.alu_src1 = AluInp.PREV_ALU_OUT
        return dp

    def _uop(seed: bool) -> UopConfig:
        u = UopConfig()
        u.inp = [InpSel.ZERO] * len(u.inp)
        u.inp_enable = [DISABLE] * len(u.inp_enable)
        u.inp[1], u.inp_enable[1] = InpSel.SRC_0, ENABLE    # lane 0 = A
        u.inp[3], u.inp_enable[3] = InpSel.SRC_1, ENABLE    # lane 2 = B
        u.inp[4], u.inp_enable[4] = InpSel.ZERO, ENABLE     # lane 3 = 0 seed
        u.out = {o: OutSel.ALU_OUT for o in OutPath}
        u.out_enable = {o: DISABLE for o in OutPath}
        if not seed:
            u.out_enable[OutPath.WR0_LO] = ENABLE
        u.datapath_config = [_dp(st, seed) for st in range(8)]
        if seed:
            # two non-consuming priming cycles write 0 into the state flop
            # (one per parity of the 2-back recurrence)
            u.repeat_count = 2
            u.trigger = (Trigger.COUNT, Trigger.NONE, Trigger.NONE)
            u.next_uop = (1, 0, 0)
            u.require_inp0 = DISABLE
            u.require_inp1 = DISABLE
        else:
            u.repeat_count = 0
            u.trigger = (Trigger.SRC_TENSOR_DONE, Trigger.NONE, Trigger.NONE)
            u.next_uop = (0, 0, 0)
            u.require_inp0 = ENABLE
            u.require_inp1 = ENABLE
        return u

    scan_uops = [_uop(seed=True), _uop(seed=False)]
    for u in scan_uops:
        u.validate("v3")

    class _Op:
        subdim = False
        perf_en: dict = {}

        def __init__(self, name, spec, uops):
            self.name = name
            self.spec = spec
            self._uops = uops

        def compile(self, ver):
            assert ver == "v3", f"custom ops only built for v3, got {ver}"
            import concourse.dve_ops as dve_ops
            key = ("compiled", self.name, ver)
            if key not in _OP_CACHE:
                uops = self._uops if self._uops is not None \
                    else lower(self.spec, ver=ver)
                _OP_CACHE[key] = DveOpSpec(
                    name=self.name,
                    opcode=dve_ops.get_dve_sub_opcode(self.name),
                    uops=uops,
                    rd1_en=True,
                )
            return _OP_CACHE[key]

    scan_op = _Op(
        "AFFINE_SCAN_2BACK_ANT",
        Spec(body=Src0 * C0 + Src1,
             reference=lambda in0, in1, s0, s1, imm2: in0 * s0 + in1),
        scan_uops,
    )
    fma_op = _Op(
        "MULT_PLUS_SRC0_ANT",
        Spec(body=Src0 * Src1 + Src0,
             reference=lambda in0, in1, s0, s1, imm2: in0 * in1 + in0),
        None,
    )
    for op in (scan_op, fma_op):
        if op.name not in dve_ops._SUB_OPCODE_FOR_NAME:
            dve_ops.OPS.append(op)
            dve_ops._SUB_OPCODE_FOR_NAME[op.name] = (
                dve_ops._CUSTOM_DVE_ROW_BASE + len(dve_ops.OPS) - 1)
            assert dve_ops._SUB_OPCODE_FOR_NAME[op.name] < 0x20
            dve_ops.CUSTOM_DVE_SPECS[op.name] = op.spec
    _OP_CACHE["ops"] = (scan_op, fma_op)
    return _OP_CACHE["ops"]


def _build_program(mu: float, alpha: float, tiles: tuple, chunks: tuple,
                   skip: int):
    import concourse.bacc as bacc
    import concourse.mybir as mybir
    from concourse.tile import TileContext

    f32 = mybir.dt.float32
    bf16 = mybir.dt.bfloat16
    AF = mybir.ActivationFunctionType
    NT = len(tiles)
    SK = skip
    assert 0 < SK <= tiles[0][1] and SK <= tiles[1][0] and SK % 2 == 0
    FMAX = 2 * max(w for _, w in tiles) + SK

    scan_op, fma_op = _get_custom_ops()

    nc = bacc.Bacc()
    in0 = nc.dram_tensor("in0", [P, C2], mybir.dt.float8e4, kind="ExternalInput")
    in1 = nc.dram_tensor("in1", [P, C2], mybir.dt.float8e4, kind="ExternalInput")
    fp8 = mybir.dt.float8e4
    aein = nc.dram_tensor("ae", [P, C2], fp8, kind="ExternalInput")
    # stats: per-tile states/merged Ln sums [0,NT), even-stream Ln sums
    # [NT,2NT) (only for split tiles), 2 end states
    out_stats = nc.dram_tensor("out_stats", [P, 2 * NT + 2], f32,
                               kind="ExternalOutput")
    out_bhead = nc.dram_tensor("out_bhead", [P, SK], f32,
                               kind="ExternalOutput")

    with TileContext(nc) as tc:
        with tc.tile_pool(name="pers", bufs=1) as pers, \
             tc.tile_pool(name="work", bufs=3) as work:
            stats = pers.tile([P, 2 * NT + 2], f32)
            musb = pers.tile([P, 1], f32)
            i0f = pers.tile([P, C2], mybir.dt.float8e4)
            i1f = pers.tile([P, C2], mybir.dt.float8e4)
            aef = pers.tile([P, C2], fp8)
            nc.gpsimd.memset(stats[:], 0.0)
            nc.gpsimd.memset(musb[:], float(mu))
            # prefetch the full input streams up front (chunked so early
            # tiles start while later chunks stream); in0/ae ride the Sync
            # ring, in1 the Scalar ring -- issued before any ACT work so
            # they are never queued behind it
            for c0, w in chunks:
                nc.sync.dma_start(i0f[:, c0:c0 + w], in0[:, c0:c0 + w])
                nc.scalar.dma_start(i1f[:, c0:c0 + w], in1[:, c0:c0 + w])
                nc.sync.dma_start(aef[:, c0:c0 + w], aein[:, c0:c0 + w])

            # dummy 1-element Ln: pulls the ~1.3us Ln ACT_TABLE_LOAD under
            # the input-DMA shadow instead of ahead of the first real Ln
            warm = pers.tile([P, 1], f32)
            nc.scalar.activation(warm[:], musb[:], AF.Ln,
                                 scale=float(alpha), bias=musb[:])

            for j, (c0, w) in enumerate(tiles):
                wu = 0 if j == 0 else SK         # warmup prefix columns
                a0 = c0 - wu
                wt = w + wu
                lo = SK if j == 0 else 0         # head skipped on device

                # bt: [0, wt) = scan states, [wt, wt+w) = even-event args
                bt = work.tile([P, FMAX], f32, tag="b")
                nc.vector._custom_dve(scan_op, out=bt[:, :wt],
                                      in0=i0f[:, a0:a0 + wt],
                                      in1=i1f[:, a0:a0 + wt])

                if j == 0:
                    # ship the carry-head block; its Ln is host-side
                    nc.scalar.dma_start(out_bhead[:], bt[:, :SK])

                # even-event lam args: xe_m = ae_m*(1 + s_{m-1}); the state
                # s_{m-1} sits at stream col m (m odd) / m-2 (m even)
                nc.vector._custom_dve(
                    fma_op, out=bt[:, wt + lo:wt + w:2],
                    in0=aef[:, c0 + lo:c0 + w:2],
                    in1=bt[:, wu + lo - 2:wu + w - 2:2])
                nc.vector._custom_dve(
                    fma_op, out=bt[:, wt + lo + 1:wt + w:2],
                    in0=aef[:, c0 + lo + 1:c0 + w:2],
                    in1=bt[:, wu + lo + 1:wu + w:2])

                lnl = work.tile([P, FMAX], bf16, tag="lnl")
                if j == 0 or w >= 1024:
                    # split: the states-half Ln depends only on the scan and
                    # overlaps the recovery on ACT; evens follow
                    nc.scalar.activation(lnl[:, :w - lo], bt[:, wu + lo:wt],
                                         AF.Ln, scale=float(alpha),
                                         bias=musb[:],
                                         accum_out=stats[:, j:j + 1])
                    nc.scalar.activation(lnl[:, w - lo:2 * (w - lo)],
                                         bt[:, wt + lo:wt + w],
                                         AF.Ln, scale=float(alpha),
                                         bias=musb[:],
                                         accum_out=stats[:, NT + j:NT + j + 1])
                else:
                    # states and even args are contiguous in bt: one Ln
                    nc.scalar.activation(lnl[:, :2 * w], bt[:, wu:wt + w],
                                         AF.Ln, scale=float(alpha),
                                         bias=musb[:],
                                         accum_out=stats[:, j:j + 1])

                if j == NT - 1:
                    # last two stream cols = states of pairs C2-1, C2-2
                    nc.vector.tensor_copy(stats[:, 2 * NT:2 * NT + 2],
                                          bt[:, wt - 2:wt])

            nc.scalar.dma_start(out_stats[:], stats[:], single_packet=True)

    nc.finalize()
    return nc


def _get_program(mu, alpha, tiles, chunks, skip):
    key = (repr(mu), repr(alpha), tuple(tiles), tuple(chunks), skip)
    prog = _PROGRAM_CACHE.get(key)
    if prog is None:
        prog = _build_program(mu, alpha, tiles, chunks, skip)
        _PROGRAM_CACHE[key] = prog
    return prog


def kernel(event_times, raw_mu, raw_alpha, raw_beta, _want_trace=False):
    from concourse.bass_utils import run_bass_kernel_spmd

    ev_full = np.ascontiguousarray(np.asarray(event_times, dtype=np.float32))
    assert ev_full.shape == (N,), ev_full.shape
    mu = _softplus64(float(np.asarray(raw_mu))) + EPS
    alpha = _softplus64(float(np.asarray(raw_alpha))) + EPS
    beta = _softplus64(float(np.asarray(raw_beta))) + EPS
    T = float(ev_full[-1])

    # a_i = exp(-beta*dt_i) over the halo-extended event array (f32 dt, f64
    # exp); index e in a_ext = global event e-H, the first H are core-0 pad.
    dt_full = np.empty(N, np.float64)
    dt_full[0] = PAD_GAP
    dt_full[1:] = np.subtract(ev_full[1:], ev_full[:-1], dtype=np.float32)
    a_ext = np.exp(-beta * dt_full)

    # Pair maps: pair q = events (2q, 2q+1):  R -> A_q R + B_q
    aep = a_ext[0::2]
    aop = a_ext[1::2]
    A = aep * aop
    B = A + aop
    # Quad-interleaved streams on the pair grid: even stream col 2r holds
    # the aligned quad (pairs 2r, 2r+1), odd col 2r+1 the straddled quad
    # (pairs 2r-1, 2r); the 2-back scan then emits the pair-chain state
    # (R at the pair's odd event) at every column, pairwise swapped.
    A_e, A_o = A[0::2], A[1::2]
    B_e, B_o = B[0::2], B[1::2]
    A_em1 = np.empty_like(A_e)                 # A[2r-1]
    A_em1[0], A_em1[1:] = 0.0, A_o[:-1]
    B_em1 = np.empty_like(B_e)                 # B[2r-1]
    B_em1[0], B_em1[1:] = 0.0, B_o[:-1]
    IN0 = np.empty(N + H >> 1, np.float32)
    IN1 = np.empty(N + H >> 1, np.float32)
    IN0[0::2] = A_e * A_o
    IN1[0::2] = A_o * B_e + B_o
    IN0[1::2] = A_e * A_em1
    IN1[1::2] = A_e * B_em1 + B_e
    IN0 = IN0.astype(ml_dtypes.float8_e4m3fn)
    IN1 = IN1.astype(ml_dtypes.float8_e4m3fn)
    AE16 = aep.astype(np.float32).astype(ml_dtypes.float8_e4m3fn)

    # Carry/warmup window: max number of events within 110/beta time units
    # ahead of any event (margin over the f32 exp underflow at ~104).
    cnt = (np.searchsorted(ev_full, ev_full + np.float32(110.0 / beta))
           - np.arange(N))
    wc_req = int(cnt.max())
    tiles, chunks = _TILES_A, _CHUNKS_A
    skip = min(-(-(wc_req + 96) // 64) * 32, tiles[0][1])
    if wc_req + 32 > 2 * skip or skip > tiles[1][0]:
        tiles, chunks = _TILES_B, _CHUNKS_B
        skip = min(-(-(wc_req + 96) // 64) * 32, tiles[0][1])
        if wc_req + 32 > 2 * skip or skip > tiles[1][0]:
            raise RuntimeError(
                f"carry window {wc_req} exceeds head tile; beta={beta} too "
                f"small for this build")

    # Per-core inputs and host-side fixup metadata
    S2, L2 = S // 2, L // 2
    in_maps = []
    t2ds = []      # per-core [P, CE] event-time windows (f64)
    for k in range(M):
        sl = slice(k * S2, k * S2 + L2)
        in_maps.append({
            "in0": np.ascontiguousarray(IN0[sl].reshape(P, C2)),
            "in1": np.ascontiguousarray(IN1[sl].reshape(P, C2)),
            "ae": np.ascontiguousarray(AE16[sl].reshape(P, C2)),
        })
        t2ds.append(ev_full[k * S:(k + 1) * S].astype(np.float64)
                    .reshape(P, CE))

    prog = _get_program(mu, alpha, tuple(tiles), tuple(chunks), skip)
    res = run_bass_kernel_spmd(prog, in_maps, list(range(M)),
                               trace=_want_trace)

    NT = len(tiles)
    SK = skip
    q = np.arange(SK)
    qcol = q + np.where(q % 2 == 1, -1, 1)       # stream col of pair q
    log_term = np.float64(0.0)
    rend = 0.0          # carry chained across partition rows AND cores
    for k in range(M):
        r = res.results[k]
        st = r["out_stats"].astype(np.float64)   # [P, 2NT+2]
        lg = st[:, 0:2 * NT]
        for j, (c0, w) in enumerate(tiles):
            if c0 + w <= H2:     # partition-0 columns of this tile = halo
                lg[0, j] = 0.0
                lg[0, NT + j] = 0.0
        log_term += lg.sum()

        # Host-side carry (f64).  The device scanned each chunk's two
        # quad-parity chains with state 0 and a zeroed straddle stub, so
        # both chains' truncation is exp(-beta*(t_{2p+1} - t_{-1}))*K1
        # with K1 = R at the chunk's predecessor event; it has decayed to
        # exactly 0 (f32) for pairs >= SK into the chunk.
        t2d = t2ds[k]
        tp1 = np.empty(P, np.float64)   # t at row event -1
        tp1[0] = t2d[0, 0] - 1.0 if k == 0 else ev_full[k * S - 1]
        flat = t2d.reshape(-1)
        tp1[1:] = flat[CE - 1:L - 1:CE]
        bend1 = st[:, 2 * NT]        # state of pair C2-1 (odd pair chain)
        K1 = np.zeros(P, np.float64)
        for p in range(P):
            K1[p] = rend
            rend = bend1[p] + np.exp(-beta * (t2d[p, CE - 1] - tp1[p])) * rend
        bhead = r["out_bhead"].astype(np.float64)    # [P, SK] stream cols
        # true pair states s(q), q in [0, SK)
        todd = t2d[:, 2 * q + 1]                     # [P, SK]
        s_true = (bhead[:, qcol]
                  + np.exp(-beta * (todd - tp1[:, None])) * K1[:, None])
        # odd events 2q+1
        ln_o = np.log(mu + alpha * s_true)
        # even events 2q: R = a*(1 + s(q-1))
        s_prev = np.empty_like(s_true)
        s_prev[:, 0] = K1
        s_prev[:, 1:] = s_true[:, :-1]
        teven = t2d[:, 2 * q]
        tprev = np.empty_like(teven)
        tprev[:, 0] = tp1
        tprev[:, 1:] = t2d[:, 1:2 * SK - 1:2]
        a_ev = np.exp(-beta * (teven - tprev))
        ln_e = np.log(mu + alpha * (a_ev * (1.0 + s_prev)))
        log_term += ln_o.sum() + ln_e.sum()

    # Integral term fully on host (f64)
    lo_i = int(np.searchsorted(ev_full, np.float32(T - 700.0 / beta)))
    int_exp = float(np.exp(-beta * (np.float64(T) -
                                    ev_full[lo_i:].astype(np.float64))).sum())
    integral_term = mu * T + (alpha / beta) * (N - int_exp)

    branching = alpha / beta
    penalty = PENALTY * max(branching - 0.999, 0.0) ** 2
    loglik = log_term - integral_term - penalty
    out = np.float32(-loglik)
    if _want_trace:
        return out, res
    return out


# revision 53
# speedup vs baseline: 1.0615x; 1.0036x over previous
"""Exponential Hawkes process negative log-likelihood on 8 Trainium2 cores.

Math (reference):
    R_0 = 0;  R_i = exp(-beta*(t_i - t_{i-1})) * (1 + R_{i-1})
    lam_i = mu + alpha * R_i
    nll = -[ sum_i log(lam_i) - mu*T - (alpha/beta) * sum_i (1 - exp(-beta*(T - t_i)))
             - 1000 * relu(alpha/beta - 0.999)^2 ]

Strategy (blocked scan, per the sharding hint):
  - The serial bottleneck is the per-event affine recurrence.  The stock
    DVE ``tensor_tensor_scan`` runs ~2 cycles/element (state routed
    backward one stage through the ``out_a`` flop with a one-cycle
    bubble).  This kernel registers a hand-written custom DVE micro-op
    program with the bubble removed: elements issue every cycle and the
    backward route delivers the state from TWO elements back.
  - The recurrence is pair-compacted on the host: pair p has the affine
    map R -> A_p R + B_p over its two events.  On the pair sequence the
    custom op is fed an interleaved stream of quad maps (aligned pair
    (2r,2r+1) at even columns, straddled (2r-1,2r) at odd columns), so
    the 2-back recurrence yields the pair-chain state (= R at every odd
    event) at 1 element/cycle -- i.e. 0.5 DVE cycles per event.
  - Even events are recovered on-device with a second (elementwise)
    custom op, lam-arg xe_m = ae_m*(1 + s_{m-1}) fused as
    Src0*Src1 + Src0, reading the shifted state stream via strided APs.
  - The recovery ops write into the tail of the scan's own tile, so one
    contiguous ACT Ln(alpha*x + mu) with accum_out covers both streams
    per column tile (halves ACT instruction + accumulator-read count).
  - Events are sharded across 8 cores with no halo: each partition chunk
    and each core scans with state 0, and the host threads the exact
    carry chain across rows AND cores in f64, recomputing the first
    events of every chunk (the device truncation decays to exactly 0 in
    f32 past a data-verified window).  Tiles > 0 re-scan a warmup window
    so tile truncation has decayed by the tile body.
  - The three fp8-e4m3 input streams (1.57 MB/core -- vs 4.2 MB for raw
    f32 event times) are prefetched into persistent SBUF via a few chunked
    DMAs issued up front, split across both DMA rings (Sync + Scalar) so
    input loads are never queued behind ACT work and outputs never
    behind input streams (the rings are in-order).
  - The integral tail sum and the final reduction are host-side f64.
"""

import numpy as np
import ml_dtypes

# Problem constants (hardcoded per task instructions).
N = 8_388_608          # total events
M = 8                  # cores
S = N // M             # events per shard (1,048,576)
H = 0                  # no halo: the host threads the carry chain across
                       # core boundaries exactly as across partition rows
L = S + H              # per-core sequence length
P = 128                # SBUF partitions
CE = L // P            # event columns per partition (8200)
C2 = CE // 2           # pair columns per partition (4100)
H2 = H // 2            # halo boundary in pair columns (512)
EPS = 1e-8
PENALTY = 1000.0
PAD_GAP = 1.0e6        # core-0 pad dt; exp(-beta*PAD_GAP) == 0 in f32

# Column tiles (start, width) on the pair-column grid.  Tiles j>0 are scanned with a SKIP-column warmup
# prefix, so SKIP <= tiles[1][0] and SKIP <= tiles[0][1].
_TILES_A = [(0, 256), (256, 256), (512, 1664), (2176, 1280), (3456, 640)]
_TILES_B = [(0, 512), (512, 1536), (2048, 1024), (3072, 1024)]
assert sum(w for _, w in _TILES_A) == C2 and sum(w for _, w in _TILES_B) == C2
# input prefetch chunk boundaries (tile-start aligned)
_CHUNKS_A = [(0, 512), (512, 1664), (2176, 1280), (3456, 640)]
_CHUNKS_B = [(0, 512), (512, 1536), (2048, 2048)]

_PROGRAM_CACHE: dict = {}
_OP_CACHE: dict = {}


def _softplus64(x: float) -> float:
    return float(np.logaddexp(0.0, np.float64(x)))


def _get_custom_ops():
    """Register (once) the two custom DVE ops:

    AFFINE_SCAN_2BACK_ANT -- hand-written micro-op program:
        out[i] = in0[i] * state + in1[i],  state = out[i-2]  (seeded 0).
    Mirrors the stock tensor_tensor_scan datapath (stage 1 MULT via the
    backward NEXT_ALU_OUT_A route, stage 2 ADD writing the out_a state
    flop) minus the stock one-element bubble, so elements issue every
    cycle and the backward route delivers the state from 2 back.

    MULT_PLUS_SRC0_ANT -- DSL-lowered elementwise fused op:
        out[i] = in0[i]*in1[i] + in0[i]
    """
    if "ops" in _OP_CACHE:
        return _OP_CACHE["ops"]

    import concourse.dve_ops as dve_ops
    from concourse.dve_spec import Spec, Src0, Src1, C0, lower
    from concourse.dve_uop import (
        UopConfig, UopDpConfig, DveOpSpec, AluOp, AluInp, InpSel, OutSel,
        OutPath, Trigger, DelayInp,
    )

    ENABLE, DISABLE = 1, 0

    def _dp(stage: int, seed: bool) -> UopDpConfig:
        dp = UopDpConfig()
        dp.delay = [DelayInp.PREV_DELAY] * 4 + [DelayInp.PREV_ALU_OUT] * 3
        dp.delay_enable = [ENABLE] * 4 + [DISABLE] * 3
        dp.alu_out_enable = ENABLE
        if stage == 1 and not seed:
            dp.op = AluOp.MULTIPLY
            dp.alu_src0 = AluInp.NEXT_ALU_OUT_A   # state: stage 2's out_a flop
            dp.alu_src1 = AluInp.PREV_DELAY_0     # A_i
        elif stage == 2:
            if seed:
                dp.op = AluOp.BYPASS              # out/out_a <- 0 (lane 3)
                dp.alu_src0 = AluInp.PREV_DELAY_3
                dp.alu_src1 = AluInp.PREV_DELAY_3
            else:
                dp.op = AluOp.ADD                 # state*A + B
                dp.alu_src0 = AluInp.PREV_ALU_OUT
                dp.alu_src1 = AluInp.PREV_DELAY_2
            dp.alu_out_a_enable = ENABLE          # state write-back
        else:
            dp.op = AluOp.BYPASS
            dp.alu_src0 = AluInp.PREV_ALU_OUT
            dp.alu_src1 = AluInp.PREV_ALU_OUT
        return dp

    def _uop(seed: bool) -> UopConfig:
        u = UopConfig()
        u.inp = [InpSel.ZERO] * len(u.inp)
        u.inp_enable = [DISABLE] * len(u.inp_enable)
        u.inp[1], u.inp_enable[1] = InpSel.SRC_0, ENABLE    # lane 0 = A
        u.inp[3], u.inp_enable[3] = InpSel.SRC_1, ENABLE    # lane 2 = B
        u.inp[4], u.inp_enable[4] = InpSel.ZERO, ENABLE     # lane 3 = 0 seed
        u.out = {o: OutSel.ALU_OUT for o in OutPath}
        u.out_enable = {o: DISABLE for o in OutPath}
        if not seed:
            u.out_enable[OutPath.WR0_LO] = ENABLE
        u.datapath_config = [_dp(st, seed) for st in range(8)]
        if seed:
            # two non-consuming priming cycles write 0 into the state flop
            # (one per parity of the 2-back recurrence)
            u.repeat_count = 2
            u.trigger = (Trigger.COUNT, Trigger.NONE, Trigger.NONE)
            u.next_uop = (1, 0, 0)
            u.require_inp0 = DISABLE
            u.require_inp1 = DISABLE
        else:
            u.repeat_count = 0
            u.trigger = (Trigger.SRC_TENSOR_DONE, Trigger.NONE, Trigger.NONE)
            u.next_uop = (0, 0, 0)
            u.require_inp0 = ENABLE
            u.require_inp1 = ENABLE
        return u

    scan_uops = [_uop(seed=True), _uop(seed=False)]
    for u in scan_uops:
        u.validate("v3")

    class _Op:
        subdim = False
        perf_en: dict = {}

        def __init__(self, name, spec, uops):
            self.name = name
            self.spec = spec
            self._uops = uops

        def compile(self, ver):
            assert ver == "v3", f"custom ops only built for v3, got {ver}"
            import concourse.dve_ops as dve_ops
            key = ("compiled", self.name, ver)
            if key not in _OP_CACHE:
                uops = self._uops if self._uops is not None \
                    else lower(self.spec, ver=ver)
                _OP_CACHE[key] = DveOpSpec(
                    name=self.name,
                    opcode=dve_ops.get_dve_sub_opcode(self.name),
                    uops=uops,
                    rd1_en=True,
                )
            return _OP_CACHE[key]

    scan_op = _Op(
        "AFFINE_SCAN_2BACK_ANT",
        Spec(body=Src0 * C0 + Src1,
             reference=lambda in0, in1, s0, s1, imm2: in0 * s0 + in1),
        scan_uops,
    )
    fma_op = _Op(
        "MULT_PLUS_SRC0_ANT",
        Spec(body=Src0 * Src1 + Src0,
             reference=lambda in0, in1, s0, s1, imm2: in0 * in1 + in0),
        None,
    )
    for op in (scan_op, fma_op):
        if op.name not in dve_ops._SUB_OPCODE_FOR_NAME:
            dve_ops.OPS.append(op)
            dve_ops._SUB_OPCODE_FOR_NAME[op.name] = (
                dve_ops._CUSTOM_DVE_ROW_BASE + len(dve_ops.OPS) - 1)
            assert dve_ops._SUB_OPCODE_FOR_NAME[op.name] < 0x20
            dve_ops.CUSTOM_DVE_SPECS[op.name] = op.spec
    _OP_CACHE["ops"] = (scan_op, fma_op)
    return _OP_CACHE["ops"]


def _build_program(mu: float, alpha: float, tiles: tuple, chunks: tuple,
                   skip: int):
    import concourse.bacc as bacc
    import concourse.mybir as mybir
    from concourse.tile import TileContext

    f32 = mybir.dt.float32
    bf16 = mybir.dt.bfloat16
    AF = mybir.ActivationFunctionType
    NT = len(tiles)
    SK = skip
    assert 0 < SK <= tiles[0][1] and SK <= tiles[1][0] and SK % 2 == 0
    FMAX = 2 * max(w for _, w in tiles) + SK

    scan_op, fma_op = _get_custom_ops()

    nc = bacc.Bacc()
    in0 = nc.dram_tensor("in0", [P, C2], mybir.dt.float8e4, kind="ExternalInput")
    in1 = nc.dram_tensor("in1", [P, C2], mybir.dt.float8e4, kind="ExternalInput")
    fp8 = mybir.dt.float8e4
    aein = nc.dram_tensor("ae", [P, C2], fp8, kind="ExternalInput")
    # stats: per-tile states/merged Ln sums [0,NT), even-stream Ln sums
    # [NT,2NT) (only for split tiles), 2 end states
    out_stats = nc.dram_tensor("out_stats", [P, 2 * NT + 2], f32,
                               kind="ExternalOutput")
    out_bhead = nc.dram_tensor("out_bhead", [P, SK], f32,
                               kind="ExternalOutput")

    with TileContext(nc) as tc:
        with tc.tile_pool(name="pers", bufs=1) as pers, \
             tc.tile_pool(name="work", bufs=3) as work:
            stats = pers.tile([P, 2 * NT + 2], f32)
            musb = pers.tile([P, 1], f32)
            i0f = pers.tile([P, C2], mybir.dt.float8e4)
            i1f = pers.tile([P, C2], mybir.dt.float8e4)
            aef = pers.tile([P, C2], fp8)
            nc.gpsimd.memset(stats[:], 0.0)
            nc.gpsimd.memset(musb[:], float(mu))
            # prefetch the full input streams up front (chunked so early
            # tiles start while later chunks stream); in0/ae ride the Sync
            # ring, in1 the Scalar ring -- issued before any ACT work so
            # they are never queued behind it
            for c0, w in chunks:
                nc.sync.dma_start(i0f[:, c0:c0 + w], in0[:, c0:c0 + w])
                nc.scalar.dma_start(i1f[:, c0:c0 + w], in1[:, c0:c0 + w])
                nc.sync.dma_start(aef[:, c0:c0 + w], aein[:, c0:c0 + w])

            # dummy 1-element Ln: pulls the ~1.3us Ln ACT_TABLE_LOAD under
            # the input-DMA shadow instead of ahead of the first real Ln
            warm = pers.tile([P, 1], f32)
            nc.scalar.activation(warm[:], musb[:], AF.Ln,
                                 scale=float(alpha), bias=musb[:])

            for j, (c0, w) in enumerate(tiles):
                wu = 0 if j == 0 else SK         # warmup prefix columns
                a0 = c0 - wu
                wt = w + wu
                lo = SK if j == 0 else 0         # head skipped on device

                # bt: [0, wt) = scan states, [wt, wt+w) = even-event args
                bt = work.tile([P, FMAX], f32, tag="b")
                nc.vector._custom_dve(scan_op, out=bt[:, :wt],
                                      in0=i0f[:, a0:a0 + wt],
                                      in1=i1f[:, a0:a0 + wt])

                if j == 0:
                    # ship the carry-head block; its Ln is host-side
                    nc.scalar.dma_start(out_bhead[:], bt[:, :SK])

                # even-event lam args: xe_m = ae_m*(1 + s_{m-1}); the state
                # s_{m-1} sits at stream col m (m odd) / m-2 (m even)
                nc.vector._custom_dve(
                    fma_op, out=bt[:, wt + lo:wt + w:2],
                    in0=aef[:, c0 + lo:c0 + w:2],
                    in1=bt[:, wu + lo - 2:wu + w - 2:2])
                nc.vector._custom_dve(
                    fma_op, out=bt[:, wt + lo + 1:wt + w:2],
                    in0=aef[:, c0 + lo + 1:c0 + w:2],
                    in1=bt[:, wu + lo + 1:wu + w:2])

                lnl = work.tile([P, FMAX], bf16, tag="lnl")
                if j == 0 or w >= 1024:
                    # split: the states-half Ln depends only on the scan and
                    # overlaps the recovery on ACT; evens follow
                    nc.scalar.activation(lnl[:, :w - lo], bt[:, wu + lo:wt],
                                         AF.Ln, scale=float(alpha),
                                         bias=musb[:],
                                         accum_out=stats[:, j:j + 1])
                    nc.scalar.activation(lnl[:, w - lo:2 * (w - lo)],
                                         bt[:, wt + lo:wt + w],
                                         AF.Ln, scale=float(alpha),
                                         bias=musb[:],
                                         accum_out=stats[:, NT + j:NT + j + 1])
                else:
                    # states and even args are contiguous in bt: one Ln
                    nc.scalar.activation(lnl[:, :2 * w], bt[:, wu:wt + w],
                                         AF.Ln, scale=float(alpha),
                                         bias=musb[:],
                                         accum_out=stats[:, j:j + 1])

                if j == NT - 1:
                    # last two stream cols = states of pairs C2-1, C2-2
                    nc.vector.tensor_copy(stats[:, 2 * NT:2 * NT + 2],
                                          bt[:, wt - 2:wt])

            nc.scalar.dma_start(out_stats[:], stats[:], single_packet=True)

    nc.finalize()
    return nc


def _get_program(mu, alpha, tiles, chunks, skip):
    key = (repr(mu), repr(alpha), tuple(tiles), tuple(chunks), skip)
    prog = _PROGRAM_CACHE.get(key)
    if prog is None:
        prog = _build_program(mu, alpha, tiles, chunks, skip)
        _PROGRAM_CACHE[key] = prog
    return prog


def kernel(event_times, raw_mu, raw_alpha, raw_beta, _want_trace=False):
    from concourse.bass_utils import run_bass_kernel_spmd

    ev_full = np.ascontiguousarray(np.asarray(event_times, dtype=np.float32))
    assert ev_full.shape == (N,), ev_full.shape
    mu = _softplus64(float(np.asarray(raw_mu))) + EPS
    alpha = _softplus64(float(np.asarray(raw_alpha))) + EPS
    beta = _softplus64(float(np.asarray(raw_beta))) + EPS
    T = float(ev_full[-1])

    # a_i = exp(-beta*dt_i) over the halo-extended event array (f32 dt, f64
    # exp); index e in a_ext = global event e-H, the first H are core-0 pad.
    dt_full = np.empty(N, np.float64)
    dt_full[0] = PAD_GAP
    dt_full[1:] = np.subtract(ev_full[1:], ev_full[:-1], dtype=np.float32)
    a_ext = np.exp(-beta * dt_full)

    # Pair maps: pair q = events (2q, 2q+1):  R -> A_q R + B_q
    aep = a_ext[0::2]
    aop = a_ext[1::2]
    A = aep * aop
    B = A + aop
    # Quad-interleaved streams on the pair grid: even stream col 2r holds
    # the aligned quad (pairs 2r, 2r+1), odd col 2r+1 the straddled quad
    # (pairs 2r-1, 2r); the 2-back scan then emits the pair-chain state
    # (R at the pair's odd event) at every column, pairwise swapped.
    A_e, A_o = A[0::2], A[1::2]
    B_e, B_o = B[0::2], B[1::2]
    A_em1 = np.empty_like(A_e)                 # A[2r-1]
    A_em1[0], A_em1[1:] = 0.0, A_o[:-1]
    B_em1 = np.empty_like(B_e)                 # B[2r-1]
    B_em1[0], B_em1[1:] = 0.0, B_o[:-1]
    IN0 = np.empty(N + H >> 1, np.float32)
    IN1 = np.empty(N + H >> 1, np.float32)
    IN0[0::2] = A_e * A_o
    IN1[0::2] = A_o * B_e + B_o
    IN0[1::2] = A_e * A_em1
    IN1[1::2] = A_e * B_em1 + B_e
    IN0 = IN0.astype(ml_dtypes.float8_e4m3fn)
    IN1 = IN1.astype(ml_dtypes.float8_e4m3fn)
    AE16 = aep.astype(np.float32).astype(ml_dtypes.float8_e4m3fn)

    # Carry/warmup window: max number of events within 110/beta time units
    # ahead of any event (margin over the f32 exp underflow at ~104).
    cnt = (np.searchsorted(ev_full, ev_full + np.float32(110.0 / beta))
           - np.arange(N))
    wc_req = int(cnt.max())
    tiles, chunks = _TILES_A, _CHUNKS_A
    skip = min(-(-(wc_req + 96) // 64) * 32, tiles[0][1])
    if wc_req + 32 > 2 * skip or skip > tiles[1][0]:
        tiles, chunks = _TILES_B, _CHUNKS_B
        skip = min(-(-(wc_req + 96) // 64) * 32, tiles[0][1])
        if wc_req + 32 > 2 * skip or skip > tiles[1][0]:
            raise RuntimeError(
                f"carry window {wc_req} exceeds head tile; beta={beta} too "
                f"small for this build")

    # Per-core inputs and host-side fixup metadata
    S2, L2 = S // 2, L // 2
    in_maps = []
    t2ds = []      # per-core [P, CE] event-time windows (f64)
    for k in range(M):
        sl = slice(k * S2, k * S2 + L2)
        in_maps.append({
            "in0": np.ascontiguousarray(IN0[sl].reshape(P, C2)),
            "in1": np.ascontiguousarray(IN1[sl].reshape(P, C2)),
            "ae": np.ascontiguousarray(AE16[sl].reshape(P, C2)),
        })
        t2ds.append(ev_full[k * S:(k + 1) * S].astype(np.float64)
                    .reshape(P, CE))

    prog = _get_program(mu, alpha, tuple(tiles), tuple(chunks), skip)
    res = run_bass_kernel_spmd(prog, in_maps, list(range(M)),
                               trace=_want_trace)

    NT = len(tiles)
    SK = skip
    q = np.arange(SK)
    qcol = q + np.where(q % 2 == 1, -1, 1)       # stream col of pair q
    log_term = np.float64(0.0)
    rend = 0.0          # carry chained across partition rows AND cores
    for k in range(M):
        r = res.results[k]
        st = r["out_stats"].astype(np.float64)   # [P, 2NT+2]
        lg = st[:, 0:2 * NT]
        for j, (c0, w) in enumerate(tiles):
            if c0 + w <= H2:     # partition-0 columns of this tile = halo
                lg[0, j] = 0.0
                lg[0, NT + j] = 0.0
        log_term += lg.sum()

        # Host-side carry (f64).  The device scanned each chunk's two
        # quad-parity chains with state 0 and a zeroed straddle stub, so
        # both chains' truncation is exp(-beta*(t_{2p+1} - t_{-1}))*K1
        # with K1 = R at the chunk's predecessor event; it has decayed to
        # exactly 0 (f32) for pairs >= SK into the chunk.
        t2d = t2ds[k]
        tp1 = np.empty(P, np.float64)   # t at row event -1
        tp1[0] = t2d[0, 0] - 1.0 if k == 0 else ev_full[k * S - 1]
        flat = t2d.reshape(-1)
        tp1[1:] = flat[CE - 1:L - 1:CE]
        bend1 = st[:, 2 * NT]        # state of pair C2-1 (odd pair chain)
        K1 = np.zeros(P, np.float64)
        for p in range(P):
            K1[p] = rend
            rend = bend1[p] + np.exp(-beta * (t2d[p, CE - 1] - tp1[p])) * rend
        bhead = r["out_bhead"].astype(np.float64)    # [P, SK] stream cols
        # true pair states s(q), q in [0, SK)
        todd = t2d[:, 2 * q + 1]                     # [P, SK]
        s_true = (bhead[:, qcol]
                  + np.exp(-beta * (todd - tp1[:, None])) * K1[:, None])
        # odd events 2q+1
        ln_o = np.log(mu + alpha * s_true)
        # even events 2q: R = a*(1 + s(q-1))
        s_prev = np.empty_like(s_true)
        s_prev[:, 0] = K1
        s_prev[:, 1:] = s_true[:, :-1]
        teven = t2d[:, 2 * q]
        tprev = np.empty_like(teven)
        tprev[:, 0] = tp1
        tprev[:, 1:] = t2d[:, 1:2 * SK - 1:2]
        a_ev = np.exp(-beta * (teven - tprev))
        ln_e = np.log(mu + alpha * (a_ev * (1.0 + s_prev)))
        log_term += ln_o.sum() + ln_e.sum()

    # Integral term fully on host (f64)
    lo_i = int(np.searchsorted(ev_full, np.float32(T - 700.0 / beta)))
    int_exp = float(np.exp(-beta * (np.float64(T) -
                                    ev_full[lo_i:].astype(np.float64))).sum())
    integral_term = mu * T + (alpha / beta) * (N - int_exp)

    branching = alpha / beta
    penalty = PENALTY * max(branching - 0.999, 0.0) ** 2
    loglik = log_term - integral_term - penalty
    out = np.float32(-loglik)
    if _want_trace:
        return out, res
    return out
